# revision 8
# baseline (speedup 1.0000x reference)
"""GroupQueryAttention Trainium2 kernel.

Sharding: 8 cores = 2 batches x 4 KV-head groups. Core c handles batch
c//4, KV head g=c%4 (i.e. q-heads 4g..4g+3, which share KV head g).
Each core computes its 4 heads' attention plus the row-parallel slice of
o_proj; the host sums the 4 partial o_proj outputs per batch (all-reduce
done on host as part of the unshard) and adds bo.

Device layout (per core):
  XT  = X[b]^T               [1024, 2048]  (hidden on partitions)
  Q^T = Wq_g^T @ XT          [256, 2048]   via lhsT=Wq_g
  KV^T= Wkv_g^T @ XT         [128, 2048]   (K^T rows 0:64, V^T rows 64:128)
  V   = PE-transpose(V^T)    16 x [128, 65] tiles, col 64 = ones
  per head h, k-tile kt, q-chunk qc(512):
    S^T  = K_kt @ Q_h^T      [128, 512] PSUM   (scale folded into Wq)
    P    = exp(S^T)          ACT -> SBUF bf16  (no max-sub: scores bounded)
    O^T += V_hat_kt^T @ P    [65, 512] PSUM    (row 64 = softmax denom)
  normalize: recip(denom) -> PE ones-broadcast -> DVE multiply
  Y_partial = O^T.T @ Wo_g   [2048, 1024] f32 -> DRAM
"""

import sys
import numpy as np

sys.path.insert(0, "/opt/trn_rl_repo")

import ml_dtypes

B, S, HID = 2, 2048, 1024
NH, NKV, D = 16, 4, 64
DG = 256          # head-dim cols per group (4 heads x 64)
N_CORES = 8
P = 128
QC = 512          # q chunk (one PSUM bank of f32)
NKT = S // P      # 16 k tiles
KH = HID // P     # 8 hidden tiles
NQC = S // QC     # 4 q chunks
SCALE = 1.0 / np.sqrt(D)

_COMPILED = {}


def _build(dt_name: str):
    import concourse.bass as bass
    import concourse.tile as tile
    from concourse import bacc, masks, mybir

    DT = getattr(mybir.dt, dt_name)
    F32 = mybir.dt.float32
    nc = bacc.Bacc("TRN2", target_bir_lowering=False, debug=False,
                   num_devices=N_CORES)

    xt_d = nc.dram_tensor("xt", [HID, S], DT, kind="ExternalInput").ap()
    wq_d = nc.dram_tensor("wq", [HID, DG], DT, kind="ExternalInput").ap()
    wkv_d = nc.dram_tensor("wkv", [HID, P], DT, kind="ExternalInput").ap()
    wo_d = nc.dram_tensor("wo", [DG, HID], DT, kind="ExternalInput").ap()
    bq_d = nc.dram_tensor("bq", [DG], F32, kind="ExternalInput").ap()
    bkv_d = nc.dram_tensor("bkv", [P], F32, kind="ExternalInput").ap()
    y_d = nc.dram_tensor("y", [S, HID], F32, kind="ExternalOutput").ap()

    Exp = mybir.ActivationFunctionType.Exp
    mult = mybir.AluOpType.mult

    with tile.TileContext(nc) as tc:
        with (
            tc.tile_pool(name="const", bufs=1) as cpool,
            tc.tile_pool(name="big", bufs=1) as big,
        ):
            ident = cpool.tile([P, P], DT)
            masks.make_identity(nc, ident[:])
            ones = cpool.tile([P, P], F32)
            nc.vector.memset(ones[:], 1.0)
            bq_t = cpool.tile([P, 2], F32)
            nc.sync.dma_start(bq_t[:], bq_d.rearrange("(a b) -> b a", b=P))
            bkv_t = cpool.tile([P, 1], F32)
            nc.sync.dma_start(bkv_t[:], bkv_d.rearrange("(a b) -> a b", b=1))

            xt = big.tile([P, KH, S], DT)
            for k in range(KH):
                nc.sync.dma_start(xt[:, k, :], xt_d[P * k:P * (k + 1), :])
            wq = big.tile([P, KH, DG], DT)
            for k in range(KH):
                nc.sync.dma_start(wq[:, k, :], wq_d[P * k:P * (k + 1), :])
            wkv = big.tile([P, KH, P], DT)
            for k in range(KH):
                nc.sync.dma_start(wkv[:, k, :], wkv_d[P * k:P * (k + 1), :])
            wo = big.tile([P, 2, HID], DT)
            for t in range(2):
                nc.sync.dma_start(wo[:, t, :], wo_d[P * t:P * (t + 1), :])

            qts = [big.tile([D, S], DT, name=f"qt{h}", tag=f"qt{h}")
                   for h in range(4)]
            kvt = big.tile([P, S], DT)       # K^T rows 0:64, V^T rows 64:128
            vhat = big.tile([P, NKT, D + 1], DT)
            nc.vector.memset(vhat[:, :, D:D + 1], 1.0)
            oT = big.tile([P, 2, S], DT)     # normalized attn out^T, packed

            # ---- projections ----
            with tc.tile_pool(name="proj_ps", bufs=4,
                              space=bass.MemorySpace.PSUM) as pps:
                for m in range(2):
                    for qc in range(NQC):
                        ps = pps.tile([P, QC], F32, tag="ps")
                        for k in range(KH):
                            nc.tensor.matmul(
                                ps[:],
                                wq[:, k, P * m:P * (m + 1)],
                                xt[:, k, QC * qc:QC * (qc + 1)],
                                start=(k == 0), stop=(k == KH - 1))
                        nc.vector.tensor_scalar_add(
                            qts[2 * m][:, QC * qc:QC * (qc + 1)],
                            ps[0:D, :], bq_t[0:D, m:m + 1])
                        nc.vector.tensor_scalar_add(
                            qts[2 * m + 1][:, QC * qc:QC * (qc + 1)],
                            ps[D:P, :], bq_t[D:P, m:m + 1])
                for qc in range(NQC):
                    ps = pps.tile([P, QC], F32, tag="ps")
                    for k in range(KH):
                        nc.tensor.matmul(
                            ps[:], wkv[:, k, :],
                            xt[:, k, QC * qc:QC * (qc + 1)],
                            start=(k == 0), stop=(k == KH - 1))
                    nc.vector.tensor_scalar_add(
                        kvt[:, QC * qc:QC * (qc + 1)], ps[:], bkv_t[:, 0:1])
                # V^T -> V (PE transpose), append ones col
                for kt in range(NKT):
                    tp = pps.tile([P, D], DT, tag="tp")
                    nc.tensor.transpose(
                        tp[:], kvt[D:P, P * kt:P * (kt + 1)], ident[D:P, D:P])
                    nc.vector.tensor_copy(vhat[:, kt, 0:D], tp[:])

            # ---- attention ----
            with (
                tc.tile_pool(name="s_ps", bufs=4,
                             space=bass.MemorySpace.PSUM) as sps,
                tc.tile_pool(name="ot_ps", bufs=4,
                             space=bass.MemorySpace.PSUM) as ops,
                tc.tile_pool(name="p_sb", bufs=6) as psb,
                tc.tile_pool(name="n_sb", bufs=4) as nsb,
            ):
                for h in range(4):
                    hp, hm = divmod(h, 2)
                    qrow = D * hm
                    ots = [ops.tile([D + 1, QC], F32, tag="ot",
                                    name=f"ot_h{h}_q{i}")
                           for i in range(NQC)]
                    for kt in range(NKT):
                        ps_list = []
                        for qc in range(NQC):
                            sp = sps.tile([P, QC], F32, tag="s")
                            nc.tensor.matmul(
                                sp[:],
                                kvt[0:D, P * kt:P * (kt + 1)],
                                qts[h][:, QC * qc:QC * (qc + 1)],
                                start=True, stop=True)
                            pt = psb.tile([P, QC], DT, tag="p")
                            nc.scalar.activation(pt[:], sp[:], Exp)
                            ps_list.append(pt)
                        for qc in range(NQC):
                            nc.tensor.matmul(
                                ots[qc][:], vhat[:, kt, :], ps_list[qc][:],
                                start=(kt == 0), stop=(kt == NKT - 1))
                    for qc in range(NQC):
                        r = nsb.tile([1, QC], F32, tag="r")
                        nc.vector.reciprocal(r[:], ots[qc][D:D + 1, :])
                        rbp = sps.tile([D, QC], F32, tag="s")
                        nc.tensor.matmul(rbp[:], ones[0:1, 0:D], r[0:1, :],
                                         start=True, stop=True)
                        rbs = nsb.tile([D, QC], F32, tag="rb")
                        nc.vector.tensor_copy(rbs[:], rbp[:])
                        nc.vector.tensor_tensor(
                            oT[qrow:qrow + D, hp, QC * qc:QC * (qc + 1)],
                            ots[qc][0:D, :], rbs[:], op=mult)

            # ---- o_proj (row-parallel partial) ----
            with (
                tc.tile_pool(name="y_ps", bufs=4,
                             space=bass.MemorySpace.PSUM) as yps,
                tc.tile_pool(name="y_sb", bufs=3) as ysb_pool,
            ):
                for tt in range(NKT):
                    ysb = ysb_pool.tile([P, HID], F32, tag="y")
                    for oc in range(2):
                        yp = yps.tile([P, QC], F32, tag="yp")
                        for ht in range(2):
                            nc.tensor.matmul(
                                yp[:],
                                oT[:, ht, P * tt:P * (tt + 1)],
                                wo[:, ht, QC * oc:QC * (oc + 1)],
                                start=(ht == 0), stop=(ht == 1))
                        nc.vector.tensor_copy(
                            ysb[:, QC * oc:QC * (oc + 1)], yp[:])
                    nc.sync.dma_start(y_d[P * tt:P * (tt + 1), :], ysb[:])

    nc.compile()
    return nc


def _get_nc(dt_name: str):
    if dt_name not in _COMPILED:
        _COMPILED[dt_name] = _build(dt_name)
    return _COMPILED[dt_name]


def kernel(X, Wq, bq, Wk, bk, Wv, bv, Wo, bo, dt_name="bfloat16",
           trace=False):
    from concourse.bass_utils import run_bass_kernel_spmd

    np_dt = ml_dtypes.bfloat16 if dt_name == "bfloat16" else np.float32
    X = np.asarray(X, np.float32)
    Wq_s = (np.asarray(Wq, np.float32) * SCALE).astype(np_dt)
    bq_s = (np.asarray(bq, np.float32) * SCALE)
    Wk = np.asarray(Wk, np.float32)
    Wv = np.asarray(Wv, np.float32)
    Wo_f = np.asarray(Wo, np.float32)

    in_maps = []
    for c in range(N_CORES):
        b, g = divmod(c, 4)
        xt = np.ascontiguousarray(X[b].T).astype(np_dt)
        wq_g = np.ascontiguousarray(Wq_s[:, DG * g:DG * (g + 1)])
        wkv_g = np.ascontiguousarray(
            np.concatenate([Wk[:, D * g:D * (g + 1)],
                            Wv[:, D * g:D * (g + 1)]], axis=1)).astype(np_dt)
        wo_g = np.ascontiguousarray(Wo_f[DG * g:DG * (g + 1), :]).astype(np_dt)
        bq_g = np.ascontiguousarray(bq_s[DG * g:DG * (g + 1)]).astype(np.float32)
        bkv_g = np.ascontiguousarray(
            np.concatenate([bk[D * g:D * (g + 1)],
                            bv[D * g:D * (g + 1)]])).astype(np.float32)
        in_maps.append({"xt": xt, "wq": wq_g, "wkv": wkv_g, "wo": wo_g,
                        "bq": bq_g, "bkv": bkv_g})

    nc = _get_nc(dt_name)
    kw = {}
    if trace:
        sys.path.insert(0, "/root/problem")
        try:
            import ntff_shim
            ntff_shim.install()
            kw["trace"] = True
        except Exception:
            pass
    res = run_bass_kernel_spmd(nc, in_maps, list(range(N_CORES)), **kw)

    out = np.zeros((B, S, HID), np.float32)
    for c in range(N_CORES):
        out[c // 4] += res.results[c]["y"]
    out += np.asarray(bo, np.float32)
    if trace:
        return out, res
    return out
